# revision 12
# baseline (speedup 1.0000x reference)
"""Trainium2 Bass kernel v3: src-sharded edges; src side via window-expand
matmuls (zero gather descriptors), dst side via dma_gather.

Math (training-mode BN makes all constant per-feature shifts cancel):
    u1  = Wc@x[src] + Wc@x[dst] + W1b@eaT      (Wc = W1a @ W_lin)
    z1  = relu(a1*u1 + c1)                      (BN1 coeffs from global stats)
    out = relu(a2*(W2@z1) + c2)                 (BN2 coeffs from global stats)

Sharding: core c owns edges with src in [c*NPC, (c+1)*NPC).  Within a core,
edges are bucketed by dst >= SPLIT (int16 gather regions) and sorted by src;
512-edge chunks are packed so each chunk's src values span < 256 nodes.  The
host stages each chunk's 256-node x window (feature-major) in `xwins`; the
device rebuilds x[src] per chunk with two one-hot expand matmuls:
    onehot[n, j] = (swin[j] == n),  swin = src - window_base (fp16, -1 = pad)
    built as:  repl = ones1.T @ swin (1-partition matmul), then DVE is_equal
    against an iota column.
Only x[dst] is fetched with dma_gather (the per-descriptor generation rate on
the GPSIMD Q7 core, ~8 ns/row, is the whole kernel's bottleneck — the src
side's descriptors are eliminated entirely).
Pads gather dedicated zero rows and have zero edge_attr and swin=-1, so padded
u1 columns are exactly 0; their effect on BN2's statistics is subtracted
analytically.  BN statistics: per-chunk bn_stats, merged, AllReduce'd.
"""

import sys
from contextlib import ExitStack

import numpy as np

try:
    import concourse  # noqa: F401
except ImportError:  # pragma: no cover
    sys.path.insert(0, "/opt/trn_rl_repo")

import ml_dtypes
from concourse import bass, bacc, mybir
from concourse import tile
from concourse.bass_utils import run_bass_kernel_spmd
from concourse.masks import make_identity

BF16 = ml_dtypes.bfloat16

N_CORES = 8
NIN = 128
EPS = 1e-5
P = 128

NPC = 6272               # src nodes per core (49 * 128)
SPLIT = 32767            # dst nodes < SPLIT are "lo", >= SPLIT are "hi"
BUCKET_ORDER = (1, 0)    # dst-hi bucket first, then dst-lo
GROUP = 2048             # edges per dst dma_gather instruction
WSPAN = 256              # max src span per 512-edge chunk (2 x 128 windows)
CHUNK = 512


def table_layout(n_nodes):
    """dst gather regions: hi = x[SPLIT:] + zero row; lo = x[0:SPLIT) + zero."""
    nhi = n_nodes - SPLIT
    hi_rows = ((nhi + 1 + 511) // 512) * 512
    lo_rows = ((SPLIT + 1 + 511) // 512) * 512
    npad = hi_rows + lo_rows
    return nhi, hi_rows, lo_rows, npad


def edge_layout(caps):
    """groups = (off, L, dst_hi); chunks = (off, gi).  caps are %512."""
    groups = []
    chunks = []
    off = 0
    for b in BUCKET_ORDER:
        dst_hi = b == 1
        rem = caps[b]
        while rem > 0:
            L = min(GROUP, rem)
            gi = len(groups)
            groups.append((off, L, dst_hi))
            for coff in range(0, L, CHUNK):
                chunks.append((off + coff, gi))
            off += L
            rem -= L
    return groups, chunks


def build_graph(n_cores, caps, n_nodes, e_total, eps=EPS):
    f32 = mybir.dt.float32
    bf16 = mybir.dt.bfloat16
    f16 = mybir.dt.float16
    i16 = mybir.dt.int16
    i32 = mybir.dt.int32
    FT = mybir.ActivationFunctionType

    nc = bacc.Bacc(
        "TRN2", target_bir_lowering=False, debug=False, num_devices=n_cores,
    )

    nhi, hi_rows, lo_rows, npad = table_layout(n_nodes)
    groups, chunksA = edge_layout(caps)
    ec = sum(caps)
    nchunk = ec // CHUNK
    n_pad_tot = ec * n_cores - e_total

    # ---- I/O -------------------------------------------------------------
    eaT = nc.dram_tensor("eaT", [P, ec], bf16, kind="ExternalInput").ap()
    xp = nc.dram_tensor("xp", [npad, P], bf16, kind="ExternalInput").ap()
    xwins = nc.dram_tensor("xwins", [P, ec // 2], bf16,
                           kind="ExternalInput").ap()
    swin = nc.dram_tensor("swin", [1, ec], f16, kind="ExternalInput").ap()
    didx = nc.dram_tensor("didx", [P, ec // 16], i16, kind="ExternalInput").ap()
    wlin = nc.dram_tensor("wlin", [P, P], f32, kind="ExternalInput").ap()
    w1 = nc.dram_tensor("w1", [P, 2 * P], f32, kind="ExternalInput").ap()
    w2 = nc.dram_tensor("w2", [P, P], f32, kind="ExternalInput").ap()
    g1 = nc.dram_tensor("g1", [P, 1], f32, kind="ExternalInput").ap()
    be1 = nc.dram_tensor("be1", [P, 1], f32, kind="ExternalInput").ap()
    g2 = nc.dram_tensor("g2", [P, 1], f32, kind="ExternalInput").ap()
    be2 = nc.dram_tensor("be2", [P, 1], f32, kind="ExternalInput").ap()
    outT = nc.dram_tensor("outT", [P, ec], bf16, kind="ExternalOutput").ap()

    grp_all = [list(range(n_cores))]

    with tile.TileContext(nc) as tc, ExitStack() as es:
        consts = es.enter_context(tc.tile_pool(name="consts", bufs=1))
        gidx = es.enter_context(tc.tile_pool(name="gidx", bufs=4))
        dram = es.enter_context(tc.tile_pool(name="dram", bufs=1, space="DRAM"))
        big = es.enter_context(tc.tile_pool(name="big", bufs=1))
        red = es.enter_context(tc.tile_pool(name="red", bufs=1))

        # ---- constants / weight prep ------------------------------------
        ident_f = consts.tile([P, P], f32)
        make_identity(nc, ident_f[:])

        wlin_s = consts.tile([P, P], f32)
        nc.sync.dma_start(out=wlin_s[:], in_=wlin)
        w1_s = consts.tile([P, 2 * P], f32)
        nc.sync.dma_start(out=w1_s[:], in_=w1)
        w2_s = consts.tile([P, P], f32)
        nc.sync.dma_start(out=w2_s[:], in_=w2)
        g1_s = consts.tile([P, 1], f32)
        nc.sync.dma_start(out=g1_s[:], in_=g1)
        be1_s = consts.tile([P, 1], f32)
        nc.sync.dma_start(out=be1_s[:], in_=be1)
        g2_s = consts.tile([P, 1], f32)
        nc.sync.dma_start(out=g2_s[:], in_=g2)
        be2_s = consts.tile([P, 1], f32)
        nc.sync.dma_start(out=be2_s[:], in_=be2)
        eps_s = consts.tile([P, 1], f32)
        nc.vector.memset(eps_s[:], eps)
        ones1 = consts.tile([1, P], f16)
        nc.vector.memset(ones1[:], 1.0)
        iota_i = consts.tile([P, 1], i32)
        nc.gpsimd.iota(iota_i[:], pattern=[[0, 1]], base=0,
                       channel_multiplier=1)
        iota_f = consts.tile([P, 1], f32)
        nc.vector.tensor_copy(iota_f[:], iota_i[:])

        # preload dst gather indices for the first groups
        idx_pre = {}
        for gi, (off, L, _dh) in enumerate(groups[:4]):
            di = gidx.tile([P, GROUP // 16], i16, tag="di")
            nc.sync.dma_start(out=di[:, :L // 16],
                              in_=didx[:, off // 16:(off + L) // 16])
            idx_pre[gi] = di

        w1aT = consts.tile([P, P], f32)
        w1bT = consts.tile([P, P], bf16)
        w2T = consts.tile([P, P], bf16)
        wcT = consts.tile([P, P], bf16)

        with tc.tile_pool(name="psum0", bufs=1, space="PSUM") as psw:
            pw = psw.tile([P, P], f32, tag="pw")
            nc.tensor.matmul(pw[:], lhsT=w1_s[:, 0:P], rhs=ident_f[:],
                             start=True, stop=True)
            nc.vector.tensor_copy(w1aT[:], pw[:])
            pw = psw.tile([P, P], f32, tag="pw")
            nc.tensor.matmul(pw[:], lhsT=w1_s[:, P:2 * P], rhs=ident_f[:],
                             start=True, stop=True)
            nc.vector.tensor_copy(w1bT[:], pw[:])
            pw = psw.tile([P, P], f32, tag="pw")
            nc.tensor.matmul(pw[:], lhsT=w2_s[:], rhs=ident_f[:],
                             start=True, stop=True)
            nc.vector.tensor_copy(w2T[:], pw[:])
            # WcT[i, o] = (W1a @ W_lin)[o, i]
            pw = psw.tile([P, P], f32, tag="pw")
            nc.tensor.matmul(pw[:], lhsT=wlin_s[:], rhs=w1aT[:],
                             start=True, stop=True)
            nc.vector.tensor_copy(wcT[:], pw[:])

        u1 = big.tile([P, ec], bf16)
        stats = consts.tile([P, nchunk, 6], f32)

        def stats_ar(tag, k0, k1):
            """Merge bn_stats entries [k0,k1) -> [P,2] sums -> AllReduce."""
            w = k1 - k0
            st = stats[:, k0:k1, :]
            se = red.tile([P, nchunk], f32, tag="se")
            nc.vector.tensor_mul(se[:, :w], st[:, :, 0], st[:, :, 1])
            so = red.tile([P, nchunk], f32, tag="so")
            nc.vector.tensor_mul(so[:, :w], st[:, :, 3], st[:, :, 4])
            qe = red.tile([P, nchunk], f32, tag="qe")
            nc.vector.tensor_mul(qe[:, :w], se[:, :w], st[:, :, 1])
            nc.vector.tensor_add(qe[:, :w], qe[:, :w], st[:, :, 2])
            qo = red.tile([P, nchunk], f32, tag="qo")
            nc.vector.tensor_mul(qo[:, :w], so[:, :w], st[:, :, 4])
            nc.vector.tensor_add(qo[:, :w], qo[:, :w], st[:, :, 5])
            nc.vector.tensor_add(se[:, :w], se[:, :w], so[:, :w])
            nc.vector.tensor_add(qe[:, :w], qe[:, :w], qo[:, :w])
            sq = red.tile([P, 2], f32, tag="sq")
            nc.vector.tensor_reduce(sq[:, 0:1], se[:, :w],
                                    axis=mybir.AxisListType.X,
                                    op=mybir.AluOpType.add)
            nc.vector.tensor_reduce(sq[:, 1:2], qe[:, :w],
                                    axis=mybir.AxisListType.X,
                                    op=mybir.AluOpType.add)
            cc_in = dram.tile([P, 2], f32, tag=f"cci{tag}")
            nc.sync.dma_start(out=cc_in[:], in_=sq[:])
            cc_out = dram.tile([P, 2], f32, tag=f"cco{tag}")
            nc.gpsimd.collective_compute(
                "AllReduce", mybir.AluOpType.add, replica_groups=grp_all,
                ins=[cc_in[:].opt()], outs=[cc_out[:].opt()])
            return cc_out

        def bn_finish(g_s, be_s, cc_parts, corr=None):
            sqg = red.tile([P, 2], f32, tag="sqg")
            nc.sync.dma_start(out=sqg[:], in_=cc_parts[0][:])
            for part in cc_parts[1:]:
                pt = red.tile([P, 2], f32, tag="sqp")
                nc.sync.dma_start(out=pt[:], in_=part[:])
                nc.vector.tensor_add(sqg[:], sqg[:], pt[:])
            if corr is not None:
                v, vq = corr
                t = red.tile([P, 2], f32, tag="tcorr")
                nc.vector.tensor_scalar_mul(t[:, 0:1], v[:], float(n_pad_tot))
                nc.vector.tensor_scalar_mul(t[:, 1:2], vq[:], float(n_pad_tot))
                nc.vector.tensor_sub(sqg[:], sqg[:], t[:])
            mu = red.tile([P, 1], f32, tag="mu")
            nc.vector.tensor_scalar_mul(mu[:], sqg[:, 0:1], 1.0 / e_total)
            var = red.tile([P, 1], f32, tag="var")
            nc.vector.tensor_scalar_mul(var[:], sqg[:, 1:2], 1.0 / e_total)
            mu2 = red.tile([P, 1], f32, tag="mu2")
            nc.vector.tensor_mul(mu2[:], mu[:], mu[:])
            nc.vector.tensor_sub(var[:], var[:], mu2[:])
            a = red.tile([P, 1], f32, tag="a")
            nc.scalar.activation(a[:], var[:], func=FT.Sqrt, bias=eps_s[:],
                                 scale=1.0)
            nc.vector.reciprocal(a[:], a[:])
            nc.vector.tensor_mul(a[:], a[:], g_s[:])
            c = red.tile([P, 1], f32, tag="c")
            nc.vector.tensor_mul(c[:], mu[:], a[:])
            nc.vector.tensor_sub(c[:], be_s[:], c[:])
            return a, c

        SLICE = 1024
        nsl = (ec + SLICE - 1) // SLICE
        preA = max(0, nchunk - 16)    # early-AR split points
        preB = max(0, nsl - 8)

        with (
            tc.tile_pool(name="psA", bufs=3, space="PSUM") as psA,
            tc.tile_pool(name="psR", bufs=2, space="PSUM") as psR,
            tc.tile_pool(name="psH", bufs=2, space="PSUM") as psH,
            tc.tile_pool(name="ea", bufs=3) as eap,
            tc.tile_pool(name="gp", bufs=4) as gp,
            tc.tile_pool(name="xw", bufs=3) as xwp,
            tc.tile_pool(name="hw", bufs=3) as hwp,
            tc.tile_pool(name="sw", bufs=3) as swp,
            tc.tile_pool(name="oh", bufs=2) as ohp,
        ):
            # ---- dst gathers --------------------------------------------
            g_tiles = {}
            for gi, (off, L, dst_hi) in enumerate(groups):
                if gi in idx_pre:
                    di = idx_pre[gi]
                else:
                    di = gidx.tile([P, GROUP // 16], i16, tag="di")
                    nc.sync.dma_start(out=di[:, :L // 16],
                                      in_=didx[:, off // 16:(off + L) // 16])
                gdst = gp.tile([P, GROUP], bf16, tag="gdst")
                dst_base = xp[0:hi_rows, :] if dst_hi else xp[hi_rows:npad, :]
                nc.gpsimd.dma_gather(
                    out_ap=gdst[:, :L].rearrange("p (a s) -> p a s", a=1),
                    in_ap=dst_base, idxs_ap=di[:, :L // 16],
                    num_idxs=L, num_idxs_reg=L, elem_size=P,
                    transpose=True, single_packet=False)
                g_tiles[gi] = (gdst, off)

            # ---- pass A --------------------------------------------------
            cc_a = []
            for k, (off, gi) in enumerate(chunksA):
                gdst, goff = g_tiles[gi]
                rel = off - goff
                S = CHUNK
                ea_t = eap.tile([P, CHUNK], bf16, tag="ea")
                nc.sync.dma_start(out=ea_t[:], in_=eaT[:, off:off + S])
                xw_t = xwp.tile([P, WSPAN], bf16, tag="xw")
                nc.sync.dma_start(out=xw_t[:],
                                  in_=xwins[:, k * WSPAN:(k + 1) * WSPAN])
                sw_t = swp.tile([1, CHUNK], f16, tag="sw")
                nc.sync.dma_start(out=sw_t[:], in_=swin[0:1, off:off + S])

                pr = psR.tile([P, CHUNK], f32, tag="pr")
                nc.tensor.matmul(pr[:], lhsT=ones1[:], rhs=sw_t[:],
                                 start=True, stop=True)
                oha = ohp.tile([P, CHUNK], bf16, tag="oha")
                nc.vector.tensor_scalar(
                    out=oha[:], in0=pr[:], scalar1=iota_f[:], scalar2=None,
                    op0=mybir.AluOpType.is_equal)
                ohb = ohp.tile([P, CHUNK], bf16, tag="ohb")
                nc.vector.tensor_scalar(
                    out=ohb[:], in0=pr[:], scalar1=128.0, scalar2=iota_f[:],
                    op0=mybir.AluOpType.subtract,
                    op1=mybir.AluOpType.is_equal)

                hp = psH.tile([P, WSPAN], f32, tag="hp")
                nc.tensor.matmul(hp[:, 0:P], lhsT=xw_t[:, 0:P], rhs=wcT[:],
                                 start=True, stop=True)
                nc.tensor.matmul(hp[:, P:WSPAN], lhsT=xw_t[:, P:WSPAN],
                                 rhs=wcT[:], start=True, stop=True)
                hw_t = hwp.tile([P, WSPAN], bf16, tag="hw")
                nc.vector.tensor_copy(hw_t[:], hp[:])

                up = psA.tile([P, CHUNK], f32, tag="up")
                nc.tensor.matmul(up[:], lhsT=hw_t[:, 0:P], rhs=oha[:],
                                 start=True, stop=False)
                nc.tensor.matmul(up[:], lhsT=hw_t[:, P:WSPAN], rhs=ohb[:],
                                 start=False, stop=False)
                nc.tensor.matmul(up[:], lhsT=wcT[:], rhs=gdst[:, rel:rel + S],
                                 start=False, stop=False)
                nc.tensor.matmul(up[:], lhsT=w1bT[:], rhs=ea_t[:],
                                 start=False, stop=True)
                nc.scalar.activation(u1[:, off:off + S], up[:],
                                     func=FT.Identity, scale=1.0)
                nc.vector.bn_stats(stats[:, k, :], u1[:, off:off + S])
            cc_a.append(stats_ar("A", 0, nchunk))

        a1, c1 = bn_finish(g1_s, be1_s, cc_a)

        # pad columns: u1 == 0 -> u2_pad = W2 @ relu(c1), constant
        rc = red.tile([P, 1], f32, tag="rc")
        nc.scalar.activation(rc[:], c1[:], func=FT.Relu)
        rcb = red.tile([P, 1], bf16, tag="rcb")
        nc.vector.tensor_copy(rcb[:], rc[:])
        with tc.tile_pool(name="psS", bufs=1, space="PSUM") as psS:
            vp = psS.tile([P, 1], f32, tag="vp")
            nc.tensor.matmul(vp[:], lhsT=w2T[:], rhs=rcb[:],
                             start=True, stop=True)
            v2 = red.tile([P, 1], f32, tag="v2")
            nc.vector.tensor_copy(v2[:], vp[:])
        v2q = red.tile([P, 1], f32, tag="v2q")
        nc.vector.tensor_mul(v2q[:], v2[:], v2[:])

        with (
            tc.tile_pool(name="psB", bufs=3, space="PSUM") as psB,
            tc.tile_pool(name="op", bufs=3) as op,
        ):
            # ---- pass B: z1 = relu(a1*u1+c1) in place; stats of W2@z1 ---
            cc_b = []
            for j in range(nsl):
                off = j * SLICE
                S = min(SLICE, ec - off)
                sl = u1[:, off:off + S]
                nc.scalar.activation(sl, sl, func=FT.Relu, scale=a1[:],
                                     bias=c1[:])
                up = psB.tile([P, SLICE], f32, tag="up")
                for h in range(0, S, CHUNK):
                    hs = min(CHUNK, S - h)
                    nc.tensor.matmul(up[:, h:h + hs], lhsT=w2T[:],
                                     rhs=u1[:, off + h:off + h + hs],
                                     start=True, stop=True)
                    nc.vector.bn_stats(stats[:, (off + h) // CHUNK, :],
                                       up[:, h:h + hs])
            cc_b.append(stats_ar("A", 0, nchunk))

            a2, c2 = bn_finish(g2_s, be2_s, cc_b, corr=(v2, v2q))

            # ---- pass C: out = relu(a2*(W2@z1)+c2) ----------------------
            for j in range(nsl):
                off = j * SLICE
                S = min(SLICE, ec - off)
                up = psB.tile([P, SLICE], f32, tag="up")
                for h in range(0, S, CHUNK):
                    hs = min(CHUNK, S - h)
                    nc.tensor.matmul(up[:, h:h + hs], lhsT=w2T[:],
                                     rhs=u1[:, off + h:off + h + hs],
                                     start=True, stop=True)
                ot = op.tile([P, SLICE], bf16, tag="ot")
                nc.scalar.activation(ot[:, :S], up[:, :S], func=FT.Relu,
                                     scale=a2[:], bias=c2[:])
                nc.sync.dma_start(out=outT[:, off:off + S], in_=ot[:, :S])

    nc.compile()
    return nc


def _wrap16(a):
    w = np.ascontiguousarray(a.reshape(-1, 16).T)
    return np.tile(w, (8, 1))


def host_prep(x, edge_index, edge_attr, n_cores):
    """Shard by src range; bucket by dst hi/lo; sort by src; pack 512-edge
    chunks with src span < WSPAN; pad to common caps."""
    n = x.shape[0]
    nhi, hi_rows, lo_rows, npad = table_layout(n)
    src_all = edge_index[0].astype(np.int64)
    dst_all = edge_index[1].astype(np.int64)

    zero_lo = SPLIT
    zero_hi = nhi

    per_core = []
    nchunks = np.zeros((n_cores, 2), np.int64)
    for c in range(n_cores):
        ids_c = np.where(src_all // NPC == c)[0]
        s, d = src_all[ids_c], dst_all[ids_c]
        key = (d >= SPLIT).astype(np.int64)
        order = np.lexsort((s, key))
        # chunk packing per bucket: <=512 edges, src span < WSPAN
        chunk_lists = {0: [], 1: []}
        for b in (0, 1):
            idx_b = order[key[order] == b]
            cur = []
            for i in idx_b:
                if cur and (len(cur) == CHUNK or
                            s[i] - s[cur[0]] >= WSPAN):
                    chunk_lists[b].append(cur)
                    cur = []
                cur.append(i)
            if cur:
                chunk_lists[b].append(cur)
        nchunks[c, 0] = len(chunk_lists[0])
        nchunks[c, 1] = len(chunk_lists[1])
        per_core.append((ids_c, s, d, chunk_lists))

    caps = tuple(int(max(1, nchunks[:, b].max())) * CHUNK for b in (0, 1))
    ec = sum(caps)
    offs = {}
    _acc = 0
    for b in BUCKET_ORDER:
        offs[b] = _acc
        _acc += caps[b]

    cores = []
    for c in range(n_cores):
        ids_c, s, d, chunk_lists = per_core[c]
        ne = len(ids_c)
        didx_p = np.empty(ec, np.int64)
        swin_p = np.full(ec, -1.0, np.float16)
        wbase = np.zeros(ec // CHUNK, np.int64)
        ea_cols = np.full(ec, -1, np.int64)  # local edge idx per padded col
        inv = np.empty(ne, np.int64)
        for b in (0, 1):
            # default pads for the whole bucket
            lo = offs[b]
            didx_p[lo:lo + caps[b]] = zero_hi if b == 1 else zero_lo
            for m, members in enumerate(chunk_lists[b]):
                coff = offs[b] + m * CHUNK
                kglob = coff // CHUNK
                sb = s[members]
                base = int(sb.min())
                assert int(sb.max()) - base < WSPAN
                wbase[kglob] = base
                pos = coff + np.arange(len(members))
                db = d[members]
                didx_p[pos] = (db - SPLIT) if b == 1 else db
                swin_p[pos] = (sb - base).astype(np.float16)
                ea_cols[pos] = members
                inv[members] = pos
        cores.append((ids_c, didx_p.astype(np.int16), swin_p, wbase,
                      ea_cols, inv))
    return caps, ec, cores, npad


def make_in_maps(x, edge_index, edge_attr, W_lin, b_lin, W1, g1, be1, W2,
                 g2, be2, n_cores):
    n = x.shape[0]
    nhi, hi_rows, lo_rows, npad = table_layout(n)
    caps, ec, cores, _ = host_prep(x, edge_index, edge_attr, n_cores)

    xbf = np.asarray(x).astype(BF16)
    xp = np.zeros((npad, P), dtype=BF16)
    xp[0:nhi] = xbf[SPLIT:n]
    xp[hi_rows:hi_rows + SPLIT] = xbf[0:SPLIT]
    # feature-major copy for window staging (zero-padded past n)
    xT = np.zeros((P, n + WSPAN), dtype=BF16)
    xT[:, :n] = xbf.T

    f32c = np.ascontiguousarray
    wlin_h = f32c(W_lin.astype(np.float32))
    w1_h = f32c(W1.astype(np.float32))
    w2_h = f32c(W2.astype(np.float32))
    g1_h = f32c(g1.astype(np.float32).reshape(P, 1))
    be1_h = f32c(be1.astype(np.float32).reshape(P, 1))
    g2_h = f32c(g2.astype(np.float32).reshape(P, 1))
    be2_h = f32c(be2.astype(np.float32).reshape(P, 1))

    groups, _ = edge_layout(caps)
    eabf = np.asarray(edge_attr).astype(BF16)

    in_maps = []
    outmaps = []
    for c in range(n_cores):
        ids_c, didx_p, swin_p, wbase, ea_cols, inv = cores[c]
        eaT = np.zeros((P, ec), dtype=BF16)
        real = ea_cols >= 0
        eaT[:, real] = eabf[ids_c[ea_cols[real]]].T
        dw = np.zeros((P, ec // 16), np.int16)
        for off, L, _dh in groups:
            dw[:, off // 16:(off + L) // 16] = _wrap16(didx_p[off:off + L])
        xwins = np.zeros((P, ec // 2), dtype=BF16)
        for kg in range(ec // CHUNK):
            b0 = int(wbase[kg])
            xwins[:, kg * WSPAN:(kg + 1) * WSPAN] = xT[:, b0:b0 + WSPAN]
        in_maps.append({
            "eaT": eaT, "xp": xp, "xwins": xwins,
            "swin": swin_p.reshape(1, ec), "didx": dw,
            "wlin": wlin_h, "w1": w1_h, "w2": w2_h,
            "g1": g1_h, "be1": be1_h, "g2": g2_h, "be2": be2_h,
        })
        outmaps.append((ids_c, inv))
    return caps, ec, in_maps, outmaps


_GRAPH_CACHE = {}


def get_graph(n_cores, caps, n_nodes, e_total):
    key = (n_cores, caps, n_nodes, e_total)
    if key not in _GRAPH_CACHE:
        _GRAPH_CACHE[key] = build_graph(n_cores, caps, n_nodes, e_total)
    return _GRAPH_CACHE[key]


def kernel(x, edge_index, edge_attr, W_lin, b_lin, W1, b1, g1, be1, W2, b2,
           g2, be2):
    x = np.asarray(x)
    edge_index = np.asarray(edge_index)
    edge_attr = np.asarray(edge_attr)
    e = edge_attr.shape[0]
    n = x.shape[0]

    caps, ec, in_maps, outmaps = make_in_maps(
        x, edge_index, edge_attr, np.asarray(W_lin), np.asarray(b_lin),
        np.asarray(W1), np.asarray(g1), np.asarray(be1), np.asarray(W2),
        np.asarray(g2), np.asarray(be2), N_CORES)
    nc = get_graph(N_CORES, caps, n, e)
    res = run_bass_kernel_spmd(nc, in_maps, core_ids=list(range(N_CORES)))
    out = np.empty((e, NIN), dtype=np.float32)
    for c in range(N_CORES):
        oT = np.asarray(res.results[c]["outT"], dtype=np.float32)
        ids_c, inv = outmaps[c]
        out[ids_c] = oT.T[inv]
    return out
